# revision 2
# baseline (speedup 1.0000x reference)
"""ConvCrossAttention Trainium2 kernel — self-contained.

Problem (B=4, C_in=C_out=256, H=W=64, N=4096):
  q = conv1x1(x1, Wq, bq); k = conv1x1(x2, Wk, bk); v = conv1x1(x2, Wv, bv)
  out = softmax(q^T k / sqrt(C)) @ v^T, back in conv layout [B, C, H, W].

Sharding: data-parallel over (batch, query-half) -> 8 NeuronCores.
Core c handles batch c//2, query rows (c%2)*2048 : (c%2+1)*2048, with the
full 4096-key context for that batch. No collectives.

Per-core program (everything SBUF-resident; ~201 us/core measured):
  Q[c, nq] = WqT.T @ X1 + bq                (PE + DVE bias)
  K[c, nk] = WkT.T @ X2 + bk
  V^T[nk, c] = X2[:, nk].T @ WvT            (V bias folded at the end)
  per 512-wide nq chunk, software-pipelined over 32 nk tiles:
    S^T[nk, nq] = K[:, t].T @ Q[:, chunk]   (PE -> PSUM, 2 K-steps)
    P = exp(S^T / 16)                       (ACT, f32r out; no max-sub
                                             needed: |scores| < ~7)
    acc[c, nq] += V^T[t, :].T @ P           (PE, PSUM accumulate)
    P-sums += P                             (Pool/DVE alternating)
  tail per chunk: den = ones.T @ P-sum (PE); bcast = ones_row.T @ (1/den);
  out = acc * bcast + bv -> DMA. Softmax normalization happens after the
  PV accumulation, so no flash-style running rescale is needed.

All matmul operands are float32r (fp32 bit layout; PE fast path at 1
cycle/row vs 4 for fp32 — ~tf32 precision, end-to-end rel err ~3e-4).
The walrus verifier requires producers feeding f32r matmuls to emit
f32r-typed outputs, hence the f32r-declared DRAM/SBUF tensors.
"""

import sys

if "/opt/trn_rl_repo" not in sys.path:
    sys.path.insert(0, "/opt/trn_rl_repo")

from contextlib import ExitStack, nullcontext

import numpy as np

import concourse.bass as bass  # noqa: F401  (engine types referenced via nc)
import concourse.mybir as mybir
import concourse.tile as tile
from concourse import bacc
from concourse.bass_utils import run_bass_kernel_spmd

F32 = mybir.dt.float32
F32R = mybir.dt.float32r

B, C, H, W = 4, 256, 64, 64
N = H * W  # 4096
NQ = 2048  # queries per core (half a batch)
NK = 4096  # full key context
CHUNK = 512
NQ_CHUNKS = NQ // CHUNK
NK_TILES = NK // 128
SCALE = 1.0 / 16.0  # C ** -0.5
PIPE = 2  # PV matmuls trail S matmuls by this many nk tiles
XDMA = 512  # input DMA chunk width


def build_nc(use_f32r=True, reps=1, loop_reps=0):
    MM = F32R if use_f32r else F32
    nc = bacc.Bacc(None, debug=False)

    x1 = nc.dram_tensor("x1c", [C, NQ], MM, kind="ExternalInput")
    x2 = nc.dram_tensor("x2c", [C, NK], MM, kind="ExternalInput")
    wq = nc.dram_tensor("wqT", [C, C], MM, kind="ExternalInput")
    wk = nc.dram_tensor("wkT", [C, C], MM, kind="ExternalInput")
    wv = nc.dram_tensor("wvT", [C, C], MM, kind="ExternalInput")
    bq = nc.dram_tensor("bq", [C, 1], F32, kind="ExternalInput")
    bk = nc.dram_tensor("bk", [C, 1], F32, kind="ExternalInput")
    bv = nc.dram_tensor("bv", [C, 1], F32, kind="ExternalInput")
    out = nc.dram_tensor("out", [C, NQ], F32, kind="ExternalOutput")

    with tile.TileContext(nc) as tc, ExitStack() as ctx:
        big = ctx.enter_context(tc.tile_pool(name="big", bufs=1))
        small = ctx.enter_context(tc.tile_pool(name="small", bufs=1))
        ppool = ctx.enter_context(tc.tile_pool(name="p", bufs=4))
        opool = ctx.enter_context(tc.tile_pool(name="o", bufs=6))
        dpool = ctx.enter_context(tc.tile_pool(name="d", bufs=2))
        spsum = ctx.enter_context(tc.tile_pool(name="spsum", bufs=2, space="PSUM"))
        apsum = ctx.enter_context(tc.tile_pool(name="apsum", bufs=4, space="PSUM"))
        dpsum = ctx.enter_context(tc.tile_pool(name="dpsum", bufs=1, space="PSUM"))

        loop_cm = tc.For_i(0, loop_reps, 1) if loop_reps else nullcontext()
        with loop_cm:
          for rep in range(reps):
            # --- weights / biases ---
            wq_sb = small.tile([128, 2, C], MM, tag="wq")
            wk_sb = small.tile([128, 2, C], MM, tag="wk")
            wv_sb = small.tile([128, 2, C], MM, tag="wv")
            for h in range(2):
                nc.sync.dma_start(out=wq_sb[:, h, :], in_=wq[h * 128 : (h + 1) * 128, :])
                nc.sync.dma_start(out=wk_sb[:, h, :], in_=wk[h * 128 : (h + 1) * 128, :])
                nc.sync.dma_start(out=wv_sb[:, h, :], in_=wv[h * 128 : (h + 1) * 128, :])
            bq_sb = small.tile([128, 2], F32, tag="bq")
            bk_sb = small.tile([128, 2], F32, tag="bk")
            bv_sb = small.tile([128, 2], F32, tag="bv")
            for h in range(2):
                nc.sync.dma_start(out=bq_sb[:, h : h + 1], in_=bq[h * 128 : (h + 1) * 128, :])
                nc.sync.dma_start(out=bk_sb[:, h : h + 1], in_=bk[h * 128 : (h + 1) * 128, :])
                nc.sync.dma_start(out=bv_sb[:, h : h + 1], in_=bv[h * 128 : (h + 1) * 128, :])
            ones_col_f32 = small.tile([128, 1], F32, tag="ones_col_f32")
            nc.vector.memset(ones_col_f32[:], 1.0)
            ones_col = small.tile([128, 1], MM, tag="ones_col")
            nc.vector.tensor_copy(ones_col[:], ones_col_f32[:])
            ones_row_f32 = small.tile([1, 128], F32, tag="ones_row_f32")
            nc.vector.memset(ones_row_f32[:], 1.0)
            ones_row = small.tile([1, 128], MM, tag="ones_row")
            nc.vector.tensor_copy(ones_row[:], ones_row_f32[:])

            # --- big SBUF residents ---
            x1_sb = [big.tile([128, NQ], MM, tag=f"x1_{h}", name=f"x1sb{h}") for h in range(2)]
            x2_sb = [big.tile([128, NK], MM, tag=f"x2_{h}", name=f"x2sb{h}") for h in range(2)]
            q_sb = [big.tile([128, NQ], MM, tag=f"q_{h}", name=f"qsb{h}") for h in range(2)]
            k_sb = [big.tile([128, NK], MM, tag=f"k_{h}", name=f"ksb{h}") for h in range(2)]
            v_sb = big.tile([128, NK_TILES, C], MM, tag="v")

            # --- load x2 chunks, project K and V^T as they arrive ---
            for j in range(NK // XDMA):
                xs_ = slice(j * XDMA, (j + 1) * XDMA)
                for h in range(2):
                    nc.sync.dma_start(
                        out=x2_sb[h][:, xs_], in_=x2[h * 128 : (h + 1) * 128, xs_]
                    )
                for sub in range(XDMA // CHUNK):
                    cs = slice(j * XDMA + sub * CHUNK, j * XDMA + (sub + 1) * CHUNK)
                    for ct in range(2):
                        kp = spsum.tile([128, CHUNK], F32, tag="s", name="kp")
                        ctslice = slice(ct * 128, (ct + 1) * 128)
                        nc.tensor.matmul(
                            kp[:], wk_sb[:, 0, ctslice], x2_sb[0][:, cs], start=True, stop=False
                        )
                        nc.tensor.matmul(
                            kp[:], wk_sb[:, 1, ctslice], x2_sb[1][:, cs], start=False, stop=True
                        )
                        nc.vector.tensor_scalar_add(
                            k_sb[ct][:, cs], kp[:], bk_sb[:, ct : ct + 1]
                        )
                for t in range(j * (XDMA // 128), (j + 1) * (XDMA // 128)):
                    ts = slice(t * 128, (t + 1) * 128)
                    vp = spsum.tile([128, C], F32, tag="s", name="vp")
                    nc.tensor.matmul(
                        vp[:], x2_sb[0][:, ts], wv_sb[:, 0, :], start=True, stop=False
                    )
                    nc.tensor.matmul(
                        vp[:], x2_sb[1][:, ts], wv_sb[:, 1, :], start=False, stop=True
                    )
                    nc.scalar.copy(v_sb[:, t, :], vp[:])

            # --- load x1, project Q ---
            for j in range(NQ // XDMA):
                xs_ = slice(j * XDMA, (j + 1) * XDMA)
                for h in range(2):
                    nc.sync.dma_start(
                        out=x1_sb[h][:, xs_], in_=x1[h * 128 : (h + 1) * 128, xs_]
                    )
                for sub in range(XDMA // CHUNK):
                    cs = slice(j * XDMA + sub * CHUNK, j * XDMA + (sub + 1) * CHUNK)
                    for ct in range(2):
                        qp = spsum.tile([128, CHUNK], F32, tag="s", name="qp")
                        ctslice = slice(ct * 128, (ct + 1) * 128)
                        nc.tensor.matmul(
                            qp[:], wq_sb[:, 0, ctslice], x1_sb[0][:, cs], start=True, stop=False
                        )
                        nc.tensor.matmul(
                            qp[:], wq_sb[:, 1, ctslice], x1_sb[1][:, cs], start=False, stop=True
                        )
                        nc.vector.tensor_scalar_add(
                            q_sb[ct][:, cs], qp[:], bq_sb[:, ct : ct + 1]
                        )

            # --- attention; each chunk's tail is emitted one chunk late so
            # the PE never stalls on the DVE reciprocal chain ---
            tail_a = tail_b = None
            for c0 in range(NQ_CHUNKS):
                cs = slice(c0 * CHUNK, (c0 + 1) * CHUNK)
                acc0 = apsum.tile([128, CHUNK], F32, tag="acc", name="acc0")
                acc1 = apsum.tile([128, CHUNK], F32, tag="acc", name="acc1")
                # P-sum split across Pool (even tiles) and DVE (odd tiles) so
                # neither engine's serial accumulation chain gates the PE.
                psum_p = dpool.tile([128, CHUNK], F32, tag="psum_p", name="psum_p")
                psum_d = dpool.tile([128, CHUNK], F32, tag="psum_d", name="psum_d")
                p_tiles = {}

                def emit_pv(t, acc0=acc0, acc1=acc1, psum_p=psum_p, psum_d=psum_d, p_tiles=p_tiles):
                    first, last = t == 0, t == NK_TILES - 1
                    p = p_tiles.pop(t)
                    nc.tensor.matmul(
                        acc0[:], v_sb[:, t, 0:128], p[:], start=first, stop=last
                    )
                    nc.tensor.matmul(
                        acc1[:], v_sb[:, t, 128:256], p[:], start=first, stop=last
                    )
                    eng, acc_ps = (nc.gpsimd, psum_p) if t % 2 == 0 else (nc.vector, psum_d)
                    if t < 2:
                        eng.tensor_copy(acc_ps[:], p[:].bitcast(F32))
                    else:
                        eng.tensor_add(acc_ps[:], acc_ps[:], p[:].bitcast(F32))

                for t in range(NK_TILES):
                    ts = slice(t * 128, (t + 1) * 128)
                    sp = spsum.tile([128, CHUNK], F32, tag="s", name="sp")
                    nc.tensor.matmul(
                        sp[:], k_sb[0][:, ts], q_sb[0][:, cs], start=True, stop=False
                    )
                    nc.tensor.matmul(
                        sp[:], k_sb[1][:, ts], q_sb[1][:, cs], start=False, stop=True
                    )
                    p = ppool.tile([128, CHUNK], MM, tag="p", name="p")
                    nc.scalar.activation(
                        p[:], sp[:], mybir.ActivationFunctionType.Exp, scale=SCALE
                    )
                    p_tiles[t] = p
                    if t >= PIPE:
                        emit_pv(t - PIPE)

                for t in range(NK_TILES - PIPE, NK_TILES):
                    emit_pv(t)
                if tail_a is not None:
                    tail_a()
                if tail_b is not None:
                    tail_b()

                def tail_a(acc0=acc0, acc1=acc1, psum_p=psum_p, psum_d=psum_d, cs=cs):
                    # denominator: one partition-reduction matmul per chunk
                    psum_acc_r = dpool.tile(
                        [128, CHUNK], MM, tag="psum_acc_r", name="psum_acc_r"
                    )
                    nc.vector.tensor_add(psum_acc_r[:], psum_p[:], psum_d[:])
                    den = dpsum.tile([1, CHUNK], F32, tag="den", name="den")
                    nc.tensor.matmul(
                        den[:], ones_col[:], psum_acc_r[:], start=True, stop=True
                    )
                    recip_f32 = dpool.tile([1, CHUNK], F32, tag="recip_f32", name="recip_f32")
                    nc.vector.reciprocal(recip_f32[:], den[:])
                    recip_sb = dpool.tile([1, CHUNK], MM, tag="recip_sb", name="recip_sb")
                    nc.vector.tensor_copy(recip_sb[:], recip_f32[:])
                    tail_a.recip_sb = recip_sb

                def tail_b(acc0=acc0, acc1=acc1, cs=cs, tail_a=tail_a):
                    recip_sb = tail_a.recip_sb
                    bcast = dpsum.tile([128, CHUNK], F32, tag="bcast", name="bcast")
                    nc.tensor.matmul(
                        bcast[:], ones_row[:], recip_sb[:], start=True, stop=True
                    )
                    bcast_sb = opool.tile([128, CHUNK], F32, tag="o", name="bcast_sb")
                    nc.vector.tensor_copy(bcast_sb[:], bcast[:])
                    for ct, acc in ((0, acc0), (1, acc1)):
                        tmp = opool.tile([128, CHUNK], F32, tag="o", name="tmp")
                        nc.vector.tensor_mul(tmp[:], acc[:], bcast_sb[:])
                        o = opool.tile([128, CHUNK], F32, tag="o", name="o")
                        nc.vector.tensor_scalar_add(o[:], tmp[:], bv_sb[:, ct : ct + 1])
                        nc.sync.dma_start(
                            out=out[ct * 128 : (ct + 1) * 128, cs], in_=o[:]
                        )

            # final chunk's tail
            tail_a()
            tail_b()

    nc.compile()
    return nc


def core_inputs(inputs, core):
    """Slice full-problem inputs for one core (numpy)."""
    b, h = core // 2, core % 2
    x1r = np.asarray(inputs["x1"], dtype=np.float32).reshape(B, C, N)
    x2r = np.asarray(inputs["x2"], dtype=np.float32).reshape(B, C, N)
    return {
        "x1c": np.ascontiguousarray(x1r[b][:, h * NQ : (h + 1) * NQ]),
        "x2c": np.ascontiguousarray(x2r[b]),
        "wqT": np.ascontiguousarray(np.asarray(inputs["Wq"], dtype=np.float32).T),
        "wkT": np.ascontiguousarray(np.asarray(inputs["Wk"], dtype=np.float32).T),
        "wvT": np.ascontiguousarray(np.asarray(inputs["Wv"], dtype=np.float32).T),
        "bq": np.asarray(inputs["bq"], dtype=np.float32).reshape(C, 1).copy(),
        "bk": np.asarray(inputs["bk"], dtype=np.float32).reshape(C, 1).copy(),
        "bv": np.asarray(inputs["bv"], dtype=np.float32).reshape(C, 1).copy(),
    }


_NC_CACHE = {}


def get_nc():
    if "nc" not in _NC_CACHE:
        _NC_CACHE["nc"] = build_nc()
    return _NC_CACHE["nc"]


def assemble(results) -> np.ndarray:
    """Gather per-core outputs into the full [4,256,64,64] f32 tensor."""
    full = np.zeros((B, C, N), np.float32)
    for core in range(8):
        b, h = core // 2, core % 2
        full[b][:, h * NQ : (h + 1) * NQ] = results[core]["out"]
    return full.reshape(B, C, H, W)


def kernel(**inputs) -> np.ndarray:
    """Full-problem entry point: full inputs in, full [4,256,64,64] f32 out."""
    nc = get_nc()
    in_maps = [core_inputs(inputs, core) for core in range(8)]
    res = run_bass_kernel_spmd(nc, in_maps, list(range(8)))
    return assemble(res.results)



# revision 4
# speedup vs baseline: 1.1752x; 1.1752x over previous
"""ConvCrossAttention Trainium2 kernel — self-contained.

Problem (B=4, C_in=C_out=256, H=W=64, N=4096):
  q = conv1x1(x1, Wq, bq); k = conv1x1(x2, Wk, bk); v = conv1x1(x2, Wv, bv)
  out = softmax(q^T k / sqrt(C)) @ v^T, back in conv layout [B, C, H, W].

Sharding: data-parallel over (batch, query-half) -> 8 NeuronCores.
Core c handles batch c//2, query rows (c%2)*2048 : (c%2+1)*2048, with the
full 4096-key context for that batch. No collectives.

Algebraic restructure (host-side weight folding only):
  softmax is invariant to per-query additive constants, so
    scores = (Wq x1 + bq)^T (Wk x2 + bk) / 16
           ~ x1^T M' x2 + alpha_m,   M' = Wq^T Wk / 16,
    alpha = u'^T x2,                 u' = Wk^T bq / 16
  (all bk terms and the bq.bk constant drop out per-query).
  So on device only the SMALL side is projected: q' = M'^T x1 (2048 cols
  vs 4096), the key side uses raw x2 as the S-matmul stationary operand,
  and alpha rides along as a 257th output column of the V projection
  (u' appended to Wv^T) -> it lands per-partition-per-key-tile, exactly
  the layout the ACT exp wants for its per-partition bias.

Per-core program (everything SBUF-resident):
  q'[c, nq]   = M'^T @ X1                   (PE, 16 MMs)
  Vx[nk, c+1] = X2[:, nk].T @ [Wv^T | u']   (PE, 64 MMs; col 256 = alpha)
  per 512-wide nq chunk, software-pipelined over 32 nk tiles:
    S^T[nk, nq] = X2[:, t].T @ q'[:, chunk] (PE -> PSUM, 2 K-steps)
    P = exp(S^T + alpha_t)                  (ACT, f32r out, bias AP;
                                             no max-sub: |scores| < ~7)
    acc[c, nq] += Vx[t, :256].T @ P         (PE, PSUM accumulate)
    P-sums += P                             (Pool/DVE alternating)
  tail per chunk (emitted one chunk late so the PE queue never stalls on
  the DVE chain): den = ones.T @ P-sum (PE); recip_approx_fast(1/den);
  bcast = ones_row.T @ (1/den) (PE); out = acc * bcast + bv -> 1 DMA.

All matmul operands are float32r (fp32 bit layout; PE fast path at 1
cycle/row vs 4 for fp32 — ~tf32 precision, end-to-end rel err ~2e-4).
Weights ship as one packed [256, 514] tensor (M' | Wv^T | u' | bv) so
startup is 2 DMAs + 12 input DMAs total, issued priority-first.
"""

import sys

if "/opt/trn_rl_repo" not in sys.path:
    sys.path.insert(0, "/opt/trn_rl_repo")

from contextlib import ExitStack

import numpy as np

import concourse.bass as bass  # noqa: F401  (engine types referenced via nc)
import concourse.mybir as mybir
import concourse.tile as tile
from concourse import bacc
from concourse.bass_utils import run_bass_kernel_spmd

F32 = mybir.dt.float32
F32R = mybir.dt.float32r

B, C, H, W = 4, 256, 64, 64
N = H * W  # 4096
NQ = 2048  # queries per core (half a batch)
NK = 4096  # full key context
CHUNK = 512
NQ_CHUNKS = NQ // CHUNK
NK_TILES = NK // 128
PIPE = 2  # PV matmuls trail S matmuls by this many nk tiles
XDMA = 512  # input DMA chunk width
WCOLS = C + (C + 2) + 1  # M' | WvT,u',pad | bv = 515  (V out width 258: fp32r needs even dst)


def build_nc():
    MM = F32R
    nc = bacc.Bacc(None, debug=False)

    xq = nc.dram_tensor("xq", [C, NQ], MM, kind="ExternalInput")
    xk = nc.dram_tensor("xk", [C, NK], MM, kind="ExternalInput")
    wp = nc.dram_tensor("wp", [C, WCOLS], MM, kind="ExternalInput")
    out = nc.dram_tensor("out", [C, NQ], F32, kind="ExternalOutput")

    with tile.TileContext(nc) as tc, ExitStack() as ctx:
        big = ctx.enter_context(tc.tile_pool(name="big", bufs=1))
        small = ctx.enter_context(tc.tile_pool(name="small", bufs=1))
        ppool = ctx.enter_context(tc.tile_pool(name="p", bufs=4))
        opool = ctx.enter_context(tc.tile_pool(name="o", bufs=4))
        dpool = ctx.enter_context(tc.tile_pool(name="d", bufs=2))
        spsum = ctx.enter_context(tc.tile_pool(name="spsum", bufs=3, space="PSUM"))
        apsum = ctx.enter_context(tc.tile_pool(name="apsum", bufs=4, space="PSUM"))
        dpsum = ctx.enter_context(tc.tile_pool(name="dpsum", bufs=1, space="PSUM"))

        # --- weights (one packed tensor, 2 DMAs) ---
        wp_sb = small.tile([128, 2, WCOLS], MM, tag="wp")
        for h in range(2):
            nc.sync.dma_start(out=wp_sb[:, h, :], in_=wp[h * 128 : (h + 1) * 128, :])
        mslice = lambda h, ct: wp_sb[:, h, ct * 128 : (ct + 1) * 128]  # noqa: E731
        wv_sl = lambda h: wp_sb[:, h, C : C + C + 2]  # noqa: E731
        bv_sl = lambda ct: wp_sb[:, 0, WCOLS - 1 : WCOLS] if ct == 0 else wp_sb[
            :, 1, WCOLS - 1 : WCOLS
        ]  # bv half ct lives in wp rows ct*128..(ct+1)*128, col 513

        ones_col_f32 = small.tile([128, 1], F32, tag="ones_col_f32")
        nc.vector.memset(ones_col_f32[:], 1.0)
        ones_col = small.tile([128, 1], MM, tag="ones_col")
        nc.vector.tensor_copy(ones_col[:], ones_col_f32[:])
        ones_row_f32 = small.tile([1, 128], F32, tag="ones_row_f32")
        nc.vector.memset(ones_row_f32[:], 1.0)
        ones_row = small.tile([1, 128], MM, tag="ones_row")
        nc.vector.tensor_copy(ones_row[:], ones_row_f32[:])

        # --- big SBUF residents ---
        x2_sb = big.tile([128, 2, NK], MM, tag="x2")
        x1_sb = big.tile([128, 2, NQ], MM, tag="x1")
        q_sb = big.tile([128, 2, NQ], MM, tag="q")
        va_sb = big.tile([128, NK_TILES, C + 2], MM, tag="va")

        # --- input DMAs, priority-first ---
        def dma_xk(j):
            cs = slice(j * XDMA, (j + 1) * XDMA)
            nc.sync.dma_start(
                out=x2_sb[:, :, cs],
                in_=xk[:, cs].rearrange("(h p) c -> p h c", h=2),
            )

        def dma_xq(j):
            cs = slice(j * XDMA, (j + 1) * XDMA)
            nc.sync.dma_start(
                out=x1_sb[:, :, cs],
                in_=xq[:, cs].rearrange("(h p) c -> p h c", h=2),
            )

        def vproj(piece):
            for t in range(piece * 4, piece * 4 + 4):
                ts = slice(t * 128, (t + 1) * 128)
                vp = spsum.tile([128, C + 2], F32, tag="s", name="vp")
                nc.tensor.matmul(
                    vp[:], x2_sb[:, 0, ts], wv_sl(0), start=True, stop=False
                )
                nc.tensor.matmul(
                    vp[:], x2_sb[:, 1, ts], wv_sl(1), start=False, stop=True
                )
                nc.scalar.copy(va_sb[:, t, :], vp[:])

        def qproj(j):
            cs = slice(j * XDMA, (j + 1) * XDMA)
            for ct in range(2):
                qp = spsum.tile([128, CHUNK], F32, tag="s", name="qp")
                nc.tensor.matmul(
                    qp[:], mslice(0, ct), x1_sb[:, 0, cs], start=True, stop=False
                )
                nc.tensor.matmul(
                    qp[:], mslice(1, ct), x1_sb[:, 1, cs], start=False, stop=True
                )
                nc.vector.tensor_copy(q_sb[:, ct, cs], qp[:])

        dma_xk(0)
        dma_xq(0)
        vproj(0)
        qproj(0)
        for j in range(1, NK // XDMA):
            dma_xk(j)
            vproj(j)
        for j in range(1, NQ // XDMA):
            dma_xq(j)
            qproj(j)

        # --- attention; each chunk's tail is emitted one chunk late so
        # the PE queue never stalls on the DVE reciprocal chain ---
        tail_a = tail_b = None
        for c0 in range(NQ_CHUNKS):
            cs = slice(c0 * CHUNK, (c0 + 1) * CHUNK)
            acc0 = apsum.tile([128, CHUNK], F32, tag="acc", name="acc0")
            acc1 = apsum.tile([128, CHUNK], F32, tag="acc", name="acc1")
            # P-sum split across Pool (even tiles) and DVE (odd tiles) so
            # neither engine's serial accumulation chain gates the PE.
            psum_p = dpool.tile([128, CHUNK], F32, tag="psum_p", name="psum_p")
            psum_d = dpool.tile([128, CHUNK], F32, tag="psum_d", name="psum_d")
            p_tiles = {}

            def emit_pv(t, acc0=acc0, acc1=acc1, psum_p=psum_p, psum_d=psum_d, p_tiles=p_tiles):
                first, last = t == 0, t == NK_TILES - 1
                p = p_tiles.pop(t)
                nc.tensor.matmul(
                    acc0[:], va_sb[:, t, 0:128], p[:], start=first, stop=last
                )
                nc.tensor.matmul(
                    acc1[:], va_sb[:, t, 128:256], p[:], start=first, stop=last
                )
                eng, acc_ps = (nc.gpsimd, psum_p) if t % 2 == 0 else (nc.vector, psum_d)
                if t < 2:
                    eng.tensor_copy(acc_ps[:], p[:].bitcast(F32))
                else:
                    eng.tensor_add(acc_ps[:], acc_ps[:], p[:].bitcast(F32))

            for t in range(NK_TILES):
                ts = slice(t * 128, (t + 1) * 128)
                sp = spsum.tile([128, CHUNK], F32, tag="s", name="sp")
                nc.tensor.matmul(
                    sp[:], x2_sb[:, 0, ts], q_sb[:, 0, cs], start=True, stop=False
                )
                nc.tensor.matmul(
                    sp[:], x2_sb[:, 1, ts], q_sb[:, 1, cs], start=False, stop=True
                )
                p = ppool.tile([128, CHUNK], MM, tag="p", name="p")
                nc.scalar.activation(
                    p[:],
                    sp[:],
                    mybir.ActivationFunctionType.Exp,
                    bias=va_sb[:, t, C : C + 1].bitcast(F32),
                )
                p_tiles[t] = p
                if t >= PIPE:
                    emit_pv(t - PIPE)

            for t in range(NK_TILES - PIPE, NK_TILES):
                emit_pv(t)
            if tail_a is not None:
                tail_a()
            if tail_b is not None:
                tail_b()

            def tail_a(acc0=acc0, acc1=acc1, psum_p=psum_p, psum_d=psum_d, cs=cs):
                # denominator: one partition-reduction matmul per chunk
                psum_acc_r = dpool.tile(
                    [128, CHUNK], MM, tag="psum_acc_r", name="psum_acc_r"
                )
                nc.vector.tensor_add(psum_acc_r[:], psum_p[:], psum_d[:])
                den = dpsum.tile([1, CHUNK], F32, tag="d", name="den")
                nc.tensor.matmul(
                    den[:], ones_col[:], psum_acc_r[:], start=True, stop=True
                )
                recip_f32 = dpool.tile([1, CHUNK], F32, tag="recip_f32", name="recip_f32")
                nc.vector.reciprocal_approx_fast(recip_f32[:], den[:])
                recip_sb = dpool.tile([1, CHUNK], MM, tag="recip_sb", name="recip_sb")
                nc.vector.tensor_copy(recip_sb[:], recip_f32[:])
                tail_a.recip_sb = recip_sb

            def tail_b(acc0=acc0, acc1=acc1, cs=cs, tail_a=tail_a):
                recip_sb = tail_a.recip_sb
                bcast = dpsum.tile([128, CHUNK], F32, tag="d", name="bcast")
                nc.tensor.matmul(
                    bcast[:], ones_row[:], recip_sb[:], start=True, stop=True
                )
                bcast_sb = opool.tile([128, CHUNK], F32, tag="ob", name="bcast_sb")
                nc.vector.tensor_copy(bcast_sb[:], bcast[:])
                o = opool.tile([128, 2, CHUNK], F32, tag="o", name="o")
                for ct, acc in ((0, acc0), (1, acc1)):
                    tmp = opool.tile([128, CHUNK], F32, tag="ob", name="tmp")
                    nc.vector.tensor_mul(tmp[:], acc[:], bcast_sb[:])
                    nc.vector.tensor_scalar_add(
                        o[:, ct, :], tmp[:], bv_sl(ct).bitcast(F32)
                    )
                nc.sync.dma_start(
                    out=out[:, cs].rearrange("(h p) c -> p h c", h=2), in_=o[:]
                )

        # final chunk's tail
        tail_a()
        tail_b()

    nc.compile()
    return nc


def core_inputs(inputs, core):
    """Slice full-problem inputs for one core (numpy)."""
    b, h = core // 2, core % 2
    x1r = np.asarray(inputs["x1"], dtype=np.float32).reshape(B, C, N)
    x2r = np.asarray(inputs["x2"], dtype=np.float32).reshape(B, C, N)
    wq = np.asarray(inputs["Wq"], dtype=np.float64)
    wk = np.asarray(inputs["Wk"], dtype=np.float64)
    wv = np.asarray(inputs["Wv"], dtype=np.float64)
    bq = np.asarray(inputs["bq"], dtype=np.float64)
    bv = np.asarray(inputs["bv"], dtype=np.float64)
    scale = 1.0 / np.sqrt(C)
    mprime = wq.T @ wk * scale  # [c1, c2]
    uprime = wk.T @ bq * scale  # [c2]
    wpack = np.concatenate(
        [mprime, wv.T, uprime[:, None], uprime[:, None], bv[:, None]], axis=1
    ).astype(np.float32)  # [256, 515]
    return {
        "xq": np.ascontiguousarray(x1r[b][:, h * NQ : (h + 1) * NQ]),
        "xk": np.ascontiguousarray(x2r[b]),
        "wp": np.ascontiguousarray(wpack),
    }


_NC_CACHE = {}


def get_nc():
    if "nc" not in _NC_CACHE:
        _NC_CACHE["nc"] = build_nc()
    return _NC_CACHE["nc"]


def assemble(results) -> np.ndarray:
    """Gather per-core outputs into the full [4,256,64,64] f32 tensor."""
    full = np.zeros((B, C, N), np.float32)
    for core in range(8):
        b, h = core // 2, core % 2
        full[b][:, h * NQ : (h + 1) * NQ] = results[core]["out"]
    return full.reshape(B, C, H, W)


def kernel(**inputs) -> np.ndarray:
    """Full-problem entry point: full inputs in, full [4,256,64,64] f32 out."""
    nc = get_nc()
    in_maps = [core_inputs(inputs, core) for core in range(8)]
    res = run_bass_kernel_spmd(nc, in_maps, list(range(8)))
    return assemble(res.results)
